# revision 11
# baseline (speedup 1.0000x reference)
"""Trainium2 Bass kernel for nn_ExplicitNReadChain (5-hop cosine attention chain
with 32k-entity decoder), 8-core SPMD.

Sharding: data-parallel over batch for the attention chain (8 batches/core);
tensor-parallel (column) over the 32k decoder entities, fed by a per-hop
AllGather of the tiny state vectors.

Per-core plan:
  - prepass: stream keys (f32), compute row L2 norms, normalize, PE-transpose
    to keysT [d, s] layout, cast fp16, keep RESIDENT in SBUF (16 MB).
  - per hop: q = state@Wq+bq (PE), l2-normalize q (ACT/DVE), logits via
    col-tiled PE matvecs against resident keysT, softmax (ACT exp w/ accum +
    DVE), attn out (f32), PE-transpose attn, weighted V read via col-tiled PE
    matvecs streaming V (fp16 HBM cache, written on hop 0), state += read.
  - after each hop: AllGather stateT across 8 cores, decode the hop's logits
    against this core's Wd column slice (E/8=4000), overlapped with the next
    hop's attention.
"""

from contextlib import ExitStack
from types import SimpleNamespace

import numpy as np

import concourse.bass as bass
import concourse.bacc as bacc
import concourse.tile as tile
import concourse.mybir as mybir
from concourse import masks

F32 = mybir.dt.float32
F16 = mybir.dt.float16
AL = mybir.AluOpType
AF = mybir.ActivationFunctionType

B, S, D, E, H = 64, 4096, 256, 32000, 5
N_CORES = 8


def _init_state(g):
    nc = g.nc
    with g.tc.tile_pool(name="init", bufs=1) as initp:
        qr0 = initp.tile([g.BC, D], F32)
        nc.sync.dma_start(qr0[:], g.q_in[:, :])
        st_ps = g.tpps.tile([128, 16], F32, tag="tp", name="st_ps")
        for dh in range(2):
            nc.tensor.transpose(st_ps[:, dh * 8:dh * 8 + g.BC],
                                qr0[:, dh * 128:(dh + 1) * 128],
                                g.ident[0:g.BC, 0:g.BC])
        nc.vector.tensor_copy(g.stateT[:], st_ps[:])


def _prepass(g):
    """Normalize keys, transpose to keysT, cast fp16, fill resident kt tiles."""
    nc = g.nc
    with (
        g.tc.tile_pool(name="knatp", bufs=2) as knatp,
        g.tc.tile_pool(name="sqp", bufs=2) as sqp,
        g.tc.tile_pool(name="invp", bufs=2) as invp,
    ):
        drain_i = 0
        for bi in range(g.BC):
            for sl in range(g.NSLAB):
                knat = knatp.tile([128, g.SLC * 256], F32, tag="knat")
                src = g.k_in[bi, sl * g.SLAB:(sl + 1) * g.SLAB, :].rearrange(
                    "(c p) d -> p c d", p=128)
                nc.sync.dma_start(knat.rearrange("p (c d) -> p c d", d=256), src)
                inv = invp.tile([128, g.SLC], F32, tag="inv")
                for c in range(g.SLC):
                    kc = knat[:, c * 256:(c + 1) * 256]
                    sq = sqp.tile([128, 256], F32, tag="sq")
                    nc.scalar.activation(sq[:], kc, AF.Square,
                                         accum_out=inv[:, c:c + 1])
                nc.scalar.sqrt(inv[:], inv[:])
                nc.vector.tensor_scalar_max(inv[:], inv[:], 1e-12)
                nc.vector.reciprocal(inv[:], inv[:])
                for c in range(g.SLC):
                    kc = knat[:, c * 256:(c + 1) * 256]
                    nc.vector.tensor_scalar_mul(kc, kc, inv[:, c:c + 1])
                for dh in range(2):
                    for cg in range(g.SLC // 4):
                        tp = g.tpps.tile([128, 512], F32, tag="tp", name="ktp_ps")
                        for ci in range(4):
                            c = cg * 4 + ci
                            nc.tensor.transpose(
                                tp[:, ci * 128:(ci + 1) * 128],
                                knat[:, c * 256 + dh * 128:c * 256 + dh * 128 + 128],
                                g.ident[:])
                        dst = g.kt[bi][dh][:, (sl * g.SLC + cg * 4) * 128:
                                           (sl * g.SLC + cg * 4 + 4) * 128]
                        nc.vector.tensor_copy(dst, tp[:])
                        drain_i += 1


def _hop_q(g, hh):
    """q = state @ Wq[hh] + bq[hh]; l2-normalize; transpose to qnT fp16."""
    nc = g.nc
    BC = g.BC
    bqt = g.scalp.tile([1, 500], F32, tag="biast", name="bqt")
    nc.scalar.dma_start(bqt[0:1, 0:D], g.bq_in[:, hh * D:(hh + 1) * D])
    q_ps = g.tpps.tile([BC, D], F32, tag="tp", name="q_ps")
    nc.tensor.matmul(q_ps[:, :], lhsT=g.ones[0:1, 0:BC], rhs=bqt[0:1, 0:D],
                     start=True, stop=False)
    for dh in range(2):
        wqt = g.wqp.tile([128, D], F32, tag="wqt")
        nc.sync.dma_start(wqt[:], g.wq_in[hh, dh])
        nc.tensor.matmul(q_ps[:, :], lhsT=g.stateT[:, dh * 8:dh * 8 + BC],
                         rhs=wqt[:], start=False, stop=(dh == 1))
    qtmp = g.scalp.tile([BC, D], F32, tag="qtmp", name="qsq")
    qss = g.scalp.tile([BC, 1], F32, tag="qss")
    nc.scalar.activation(qtmp[:], q_ps[:], AF.Square, accum_out=qss[:])
    nc.scalar.sqrt(qss[:], qss[:])
    nc.vector.tensor_scalar_max(qss[:], qss[:], 1e-12)
    nc.vector.reciprocal(qss[:], qss[:])
    qrows = g.scalp.tile([BC, D], F32, tag="qtmp", name="qrows")
    nc.vector.tensor_scalar_mul(qrows[:], q_ps[:], qss[:])
    qt_ps = g.tpps.tile([128, 16], F32, tag="tp", name="qt_ps")
    for dh in range(2):
        nc.tensor.transpose(qt_ps[:, dh * 8:dh * 8 + BC],
                            qrows[:, dh * 128:(dh + 1) * 128],
                            g.ident[0:BC, 0:BC])
    nc.vector.tensor_copy(g.qnT[:], qt_ps[:])
    # replicate each q column 32x so logits matmuls write full 128-partition
    # PSUM tiles (M=32 per col-group)
    for c in range(16):
        nc.vector.tensor_copy(g.qrep[:, c * 32:(c + 1) * 32],
                              g.qnT[:, c:c + 1].broadcast_to((128, 32)))


def _hop_logits_softmax(g, hh, pa, at):
    """One 4-batch pass: col-tiled logits matvecs, softmax, attn out+transpose."""
    nc = g.nc
    s = g.s
    expb = g.expp.tile([128, s], F32, tag="expb")
    parts = g.scalp.tile([128, g.NT], F32, tag="parts")
    for st in range(g.NT):
        lg = g.lgps.tile([128, 512], F32, tag="lg")
        for j in range(4):
            bi = pa * 4 + j
            for dh in range(2):
                nc.tensor.matmul(
                    lg[32 * j:32 * j + 32, :],
                    lhsT=g.qrep[:, (dh * 8 + bi) * 32:(dh * 8 + bi) * 32 + 32],
                    rhs=g.kt[bi][dh][:, st * 512:(st + 1) * 512],
                    start=(dh == 0), stop=(dh == 1),
                    tile_position=(0, 32 * j), skip_group_check=True)
        nc.scalar.activation(expb[:, st * 512:(st + 1) * 512], lg[:], AF.Exp,
                             accum_out=parts[:, st:st + 1])
    den = g.scalp.tile([128, 1], F32, tag="den")
    nc.vector.tensor_reduce(den[:], parts[:, 0:g.NT],
                            axis=mybir.AxisListType.X, op=AL.add)
    nc.vector.reciprocal(den[:], den[:])
    nc.vector.tensor_scalar_mul(expb[:], expb[:], den[:])
    for j in range(4):
        bi = pa * 4 + j
        nc.scalar.dma_start(g.attn_out[hh, bi, :], expb[32 * j:32 * j + 1, :])
    atv = at.rearrange("p (sc e) -> p sc e", e=8)
    for sc in range(g.SC):
        tp = g.tpps.tile([128, 128], F32, tag="tp", name="attp")
        nc.tensor.transpose(tp[:], expb[:, sc * 128:(sc + 1) * 128], g.ident[:])
        tpv = tp.rearrange("p (j r) -> p j r", j=4)
        if sc % 2 == 0:
            nc.vector.tensor_copy(atv[:, sc, pa * 4:pa * 4 + 4], tpv[:, :, 0])
        else:
            nc.scalar.copy(atv[:, sc, pa * 4:pa * 4 + 4], tpv[:, :, 0])


def _hop_vread(g, hh, pa, at):
    """Weighted V read for one 4-batch pass; update stateT."""
    nc = g.nc
    rd = g.rdps.tile([128, 256], F32, tag="rd")
    for vt_i in range(g.NVT):
        for jp in range(2):
            vts = []
            for j2 in range(2):
                j = jp * 2 + j2
                bi = pa * 4 + j
                vt = g.vp.tile([128, g.VCI * 256], F16, tag="vt")
                if hh == 0:
                    src = g.v_in[bi, vt_i * g.VCI * 128:(vt_i + 1) * g.VCI * 128, :]
                    src = src.rearrange("(c p) d -> p c d", p=128)
                    nc.gpsimd.dma_start(
                        vt.rearrange("p (c d) -> p c d", d=256), src)
                    nc.sync.dma_start(g.vcache[bi, vt_i], vt[:])
                else:
                    nc.sync.dma_start(vt[:], g.vcache[bi, vt_i])
                vts.append(vt)
            for ci in range(g.VCI):
                sc = vt_i * g.VCI + ci
                for j2 in range(2):
                    j = jp * 2 + j2
                    bi = pa * 4 + j
                    nc.tensor.matmul(
                        rd[32 * j:32 * j + 1, :],
                        lhsT=at[:, sc * 8 + bi:sc * 8 + bi + 1],
                        rhs=vts[j2][:, ci * 256:(ci + 1) * 256],
                        start=(sc == 0), stop=(sc == g.SC - 1),
                        tile_position=(0, 32 * j), skip_group_check=True)
    rdrows = g.rdrows
    for j in range(4):
        nc.vector.tensor_copy(rdrows[32 * j:32 * j + 1, :], rd[32 * j:32 * j + 1, :])
    rt = g.tpps.tile([128, 256], F32, tag="tp", name="rt")
    for dh in range(2):
        nc.tensor.transpose(rt[:, dh * 128:(dh + 1) * 128],
                            rdrows[:, dh * 128:(dh + 1) * 128], g.ident[:])
    rtv = rt.rearrange("p (dh j r) -> p dh j r", dh=2, j=4)
    for dh in range(2):
        sl = g.stateT[:, dh * 8 + pa * 4:dh * 8 + pa * 4 + 4]
        nc.vector.tensor_tensor(sl, sl, rtv[:, dh, :, 0], op=AL.add)


def _hop_decode(g, hh):
    """AllGather states; decode hop hh logits for this core's E slice."""
    nc = g.nc
    b = g.b
    nc.sync.dma_start(g.cins[hh][:], g.stateT[:])
    if g.collective and g.n_cores > 1:
        nc.gpsimd.collective_compute(
            "AllGather", AL.bypass,
            replica_groups=[list(range(g.n_cores))],
            ins=[g.cins[hh][:]], outs=[g.couts[hh][:]])
    else:
        nc.sync.dma_start(g.couts[hh][0], g.cins[hh][:])
    gst = g.gstp.tile([128, 2 * b], F32, tag="gst")
    gstv = gst.rearrange("p (dh r j) -> p dh r j", dh=2, r=g.n_cores)
    srcv = g.couts[hh].rearrange("r p (dh j) -> p r dh j", dh=2)
    for dh in range(2):
        nc.gpsimd.dma_start(gstv[:, dh], srcv[:, :, dh])
    for t in range(g.TD):
        bdt = g.scalp.tile([1, 500], F32, tag="biast", name="bdt")
        nc.scalar.dma_start(bdt[:], g.bd_in[:, hh * g.EC + t * 500:
                                            hh * g.EC + (t + 1) * 500])
        dec = g.decps.tile([128, 512], F32, tag="dec")
        nc.tensor.matmul(dec[0:b, 0:500], lhsT=g.ones[0:1, 0:b], rhs=bdt[:],
                         start=True, stop=False)
        for dh in range(2):
            wdt = g.wdp.tile([128, 500], F32, tag="wdt")
            nc.scalar.dma_start(wdt[:], g.wd_in[hh, dh, t])
            nc.tensor.matmul(dec[0:b, 0:500], lhsT=gst[:, dh * b:dh * b + b],
                             rhs=wdt[:], start=False, stop=(dh == 1))
        dout = g.outp.tile([b, 500], F32, tag="dout")
        nc.vector.tensor_copy(dout[:], dec[0:b, 0:500])
        nc.scalar.dma_start(g.logits_out[hh, :, t * 500:(t + 1) * 500], dout[:])


def _final_state(g):
    nc = g.nc
    so_ps = g.tpps.tile([16, 128], F32, tag="tp", name="so_ps")
    nc.tensor.transpose(so_ps[:], g.stateT[:], g.ident[:])
    ss = g.outp.tile([16, 128], F32, tag="dout", name="ss")
    nc.vector.tensor_copy(ss[:], so_ps[:])
    for dh in range(2):
        nc.scalar.dma_start(g.state_out[:, dh * 128:(dh + 1) * 128],
                            ss[dh * 8:dh * 8 + g.BC, :])


def build_program(n_cores=N_CORES, b=B, s=S, e=E, h_hops=H, collective=True, stage="full"):
    g = SimpleNamespace()
    g.n_cores, g.b, g.s, g.e, g.h_hops, g.collective = n_cores, b, s, e, h_hops, collective
    g.BC = b // n_cores           # batches per core
    g.EC = e // n_cores           # decoder entities per core
    g.SC = s // 128               # 128-row s-chunks
    g.NT = s // 512               # 512-wide logits tiles
    g.TD = g.EC // 500            # decoder n-tiles
    g.VCI = min(16, g.SC)         # 256-wide V chunks per V tile
    g.NVT = g.SC // g.VCI         # V tiles per batch
    g.SLAB = min(s, 2048)         # prepass slab (s rows)
    g.NSLAB = s // g.SLAB
    g.SLC = g.SLAB // 128         # chunks per slab

    nc = bacc.Bacc("TRN2", target_bir_lowering=False, debug=False,
                   num_devices=n_cores)
    g.nc = nc

    g.q_in = nc.dram_tensor("q_in", [g.BC, D], F32, kind="ExternalInput").ap()
    g.k_in = nc.dram_tensor("k_in", [g.BC, s, D], F32, kind="ExternalInput").ap()
    g.v_in = nc.dram_tensor("v_in", [g.BC, s, D], F32, kind="ExternalInput").ap()
    g.wq_in = nc.dram_tensor("wq_in", [h_hops, 2, 128, D], F32, kind="ExternalInput").ap()
    g.bq_in = nc.dram_tensor("bq_in", [1, h_hops * D], F32, kind="ExternalInput").ap()
    g.wd_in = nc.dram_tensor("wd_in", [h_hops, 2, g.TD, 128, 500], F32, kind="ExternalInput").ap()
    g.bd_in = nc.dram_tensor("bd_in", [1, h_hops * g.EC], F32, kind="ExternalInput").ap()
    g.state_out = nc.dram_tensor("state_out", [g.BC, D], F32, kind="ExternalOutput").ap()
    g.attn_out = nc.dram_tensor("attn_out", [h_hops, g.BC, s], F32, kind="ExternalOutput").ap()
    g.logits_out = nc.dram_tensor("logits_out", [h_hops, b, g.EC], F32, kind="ExternalOutput").ap()

    with tile.TileContext(nc) as tc, ExitStack() as ctx:
        g.tc = tc
        consts = ctx.enter_context(tc.tile_pool(name="consts", bufs=1))
        ktp = ctx.enter_context(tc.tile_pool(name="ktp", bufs=1))
        statep = ctx.enter_context(tc.tile_pool(name="statep", bufs=1))
        dram = ctx.enter_context(tc.tile_pool(name="dram", bufs=1, space="DRAM"))
        g.lgps = ctx.enter_context(tc.tile_pool(name="lgps", bufs=3, space="PSUM"))
        g.tpps = ctx.enter_context(tc.tile_pool(name="tpps", bufs=2, space="PSUM"))
        g.rdps = ctx.enter_context(tc.tile_pool(name="rdps", bufs=2, space="PSUM"))
        g.decps = ctx.enter_context(tc.tile_pool(name="decps", bufs=1, space="PSUM"))

        g.ident = consts.tile([128, 128], F32, name="ident")
        masks.make_identity(nc, g.ident[:])
        g.ones = consts.tile([1, 64], F32, name="ones")
        nc.gpsimd.memset(g.ones[:], 1.0)

        g.kt = [[ktp.tile([128, s], F16, name=f"kt_{bi}_{dh}", tag=f"kt_{bi}_{dh}")
                 for dh in range(2)] for bi in range(g.BC)]
        g.stateT = statep.tile([128, 16], F32, name="stateT")
        g.qnT = statep.tile([128, 16], F16, name="qnT")
        g.qrep = statep.tile([128, 512], F16, name="qrep")
        g.rdrows = statep.tile([128, 256], F32, name="rdrows")
        nc.vector.memset(g.rdrows[:], 0.0)

        g.vcache = dram.tile([g.BC, g.NVT, 128, g.VCI * 256], F16, name="vcache")
        g.cins = [dram.tile([128, 16], F32, name=f"cin{hh}", tag=f"cin{hh}")
                  for hh in range(h_hops)]
        g.couts = [dram.tile([n_cores, 128, 16], F32, name=f"cout{hh}",
                             tag=f"cout{hh}",
                             addr_space="Shared" if collective else "Local")
                   for hh in range(h_hops)]

        # one-time PSUM init: ops below read full tiles whose unused lanes
        # are never written; mark every PSUM byte initialized once.
        for pool, tag, shape, n in ((g.lgps, "lg", [128, 512], 3),
                                    (g.tpps, "tp", [128, 512], 2),
                                    (g.rdps, "rd", [128, 256], 2),
                                    (g.decps, "dec", [128, 512], 1)):
            for _ in range(n):
                t = pool.tile(shape, F32, tag=tag, name="pz")
                nc.vector.memset(t[:], 0.0)

        _init_state(g)
        _prepass(g)

        g.wqp = ctx.enter_context(tc.tile_pool(name="wqp", bufs=3))
        g.scalp = ctx.enter_context(tc.tile_pool(name="scalp", bufs=2))
        g.expp = ctx.enter_context(tc.tile_pool(name="expp", bufs=2))
        g.atp = ctx.enter_context(tc.tile_pool(name="atp", bufs=2))
        g.vp = ctx.enter_context(tc.tile_pool(name="vp", bufs=3))
        g.wdp = ctx.enter_context(tc.tile_pool(name="wdp", bufs=2))
        g.gstp = ctx.enter_context(tc.tile_pool(name="gstp", bufs=2))
        g.outp = ctx.enter_context(tc.tile_pool(name="op", bufs=2))

        if stage != "prepass":
            for hh in range(h_hops):
                _hop_q(g, hh)
                at = g.atp.tile([128, g.SC * 8], F16, tag="at", name="at")
                for pa in range(2):
                    _hop_logits_softmax(g, hh, pa, at)
                if stage == "logits":
                    continue
                for pa in range(2):
                    _hop_vread(g, hh, pa, at)
                if stage == "vread":
                    continue
                _hop_decode(g, hh)
        _final_state(g)

    nc.compile()
    return nc


# ---------------------------------------------------------------------------
# host wrapper
# ---------------------------------------------------------------------------

_RUNNER = {}


def make_in_maps(query, keys, values, Wq, bq, Wd, bd, n_cores=N_CORES,
                 b=B, e=E, h_hops=H):
    BC = b // n_cores
    EC = e // n_cores
    TD = EC // 500
    wq_h = np.ascontiguousarray(Wq.reshape(h_hops, 2, 128, D))
    bq_h = np.ascontiguousarray(bq.reshape(1, h_hops * D))
    in_maps = []
    for c in range(n_cores):
        bs = slice(c * BC, (c + 1) * BC)
        es = slice(c * EC, (c + 1) * EC)
        wd_c = Wd[:, :, es].reshape(h_hops, 2, 128, TD, 500).transpose(0, 1, 3, 2, 4)
        in_maps.append({
            "q_in": np.ascontiguousarray(query[bs]),
            "k_in": np.ascontiguousarray(keys[bs]),
            "v_in": np.ascontiguousarray(values[bs]),
            "wq_in": wq_h,
            "bq_in": bq_h,
            "wd_in": np.ascontiguousarray(wd_c),
            "bd_in": np.ascontiguousarray(bd[:, es].reshape(1, h_hops * EC)),
        })
    return in_maps


def assemble(results):
    state = np.concatenate([r["state_out"] for r in results], axis=0)
    attn = np.concatenate([r["attn_out"] for r in results], axis=1)
    logits = np.concatenate([r["logits_out"] for r in results], axis=2)
    return state, logits, attn


def get_runner():
    """Build+compile the program once; return a callable in_maps -> results."""
    if "fn" in _RUNNER:
        return _RUNNER["fn"]
    import jax
    from jax.sharding import Mesh, PartitionSpec
    from jax.experimental.shard_map import shard_map
    from concourse import bass2jax

    nc = build_program()
    bass2jax.install_neuronx_cc_hook()

    partition_name = (nc.partition_id_tensor.name
                      if nc.partition_id_tensor else None)
    in_names, out_names, out_avals, zero_outs = [], [], [], []
    for alloc in nc.m.functions[0].allocations:
        if not isinstance(alloc, mybir.MemoryLocationSet):
            continue
        name = alloc.memorylocations[0].name
        if alloc.kind == "ExternalInput":
            if name != partition_name:
                in_names.append(name)
        elif alloc.kind == "ExternalOutput":
            out_names.append(name)
            shape = tuple(alloc.tensor_shape)
            dtype = mybir.dt.np(alloc.dtype)
            out_avals.append(jax.core.ShapedArray(shape, dtype))
            zero_outs.append(np.zeros(shape, dtype))
    n_params = len(in_names)
    all_in_names = in_names + out_names
    if partition_name is not None:
        all_in_names = all_in_names + [partition_name]

    def _body(*args):
        operands = list(args)
        if partition_name is not None:
            operands.append(bass2jax.partition_id_tensor())
        outs = bass2jax._bass_exec_p.bind(
            *operands,
            out_avals=tuple(out_avals),
            in_names=tuple(all_in_names),
            out_names=tuple(out_names),
            lowering_input_output_aliases=(),
            sim_require_finite=False,
            sim_require_nnan=False,
            nc=nc,
        )
        return tuple(outs)

    devices = jax.devices()[:N_CORES]
    mesh = Mesh(np.asarray(devices), ("core",))
    n_outs = len(out_names)
    sharded = jax.jit(
        shard_map(_body, mesh=mesh,
                  in_specs=(PartitionSpec("core"),) * (n_params + n_outs),
                  out_specs=(PartitionSpec("core"),) * n_outs,
                  check_rep=False),
        donate_argnums=tuple(range(n_params, n_params + n_outs)),
        keep_unused=True,
    )

    def fn(in_maps):
        concat_in = [
            np.concatenate([np.asarray(in_maps[c][nm]) for c in range(N_CORES)], axis=0)
            for nm in in_names
        ]
        concat_zeros = [
            np.zeros((N_CORES * z.shape[0], *z.shape[1:]), z.dtype) for z in zero_outs
        ]
        out_arrs = sharded(*concat_in, *concat_zeros)
        return [
            {nm: np.asarray(out_arrs[i]).reshape(N_CORES, *out_avals[i].shape)[c]
             for i, nm in enumerate(out_names)}
            for c in range(N_CORES)
        ]

    fn.sharded = sharded
    fn.in_names = in_names
    fn.out_names = out_names
    fn.out_avals = out_avals
    fn.zero_outs = zero_outs
    fn.mesh = mesh
    _RUNNER["fn"] = fn
    return fn


def kernel(query, keys, values, Wq, bq, Wd, bd):
    query, keys, values, Wq, bq, Wd, bd = [
        np.asarray(x, dtype=np.float32)
        for x in (query, keys, values, Wq, bq, Wd, bd)
    ]
    fn = get_runner()
    results = fn(make_in_maps(query, keys, values, Wq, bq, Wd, bd))
    return assemble(results)


# revision 15
# speedup vs baseline: 1.7121x; 1.7121x over previous
"""Trainium2 Bass kernel for nn_ExplicitNReadChain (5-hop cosine attention chain
with 32k-entity decoder), 8-core SPMD.

Sharding: data-parallel over batch for the attention chain (8 batches/core);
tensor-parallel (column) over the 32k decoder entities, fed by a per-hop
AllGather of the tiny state vectors.

Per-core plan:
  - prepass: stream keys (f32), compute row L2 norms, normalize, PE-transpose
    to keysT [d, s] layout, cast fp16, keep RESIDENT in SBUF (16 MB).
  - per hop: q = state@Wq+bq (PE), l2-normalize q (ACT/DVE), logits via
    col-tiled PE matvecs against resident keysT, softmax (ACT exp w/ accum +
    DVE), attn out (f32), PE-transpose attn, weighted V read via col-tiled PE
    matvecs streaming V (fp16 HBM cache, written on hop 0), state += read.
  - after each hop: AllGather stateT across 8 cores, decode the hop's logits
    against this core's Wd column slice (E/8=4000), overlapped with the next
    hop's attention.
"""

from contextlib import ExitStack
from types import SimpleNamespace

import numpy as np

import concourse.bass as bass
import concourse.bacc as bacc
import concourse.tile as tile
import concourse.mybir as mybir
from concourse import masks

F32 = mybir.dt.float32
F16 = mybir.dt.float16
AL = mybir.AluOpType
AF = mybir.ActivationFunctionType

B, S, D, E, H = 64, 4096, 256, 32000, 5
N_CORES = 8


def _init_state(g):
    nc = g.nc
    with g.tc.tile_pool(name="init", bufs=1) as initp:
        qr0 = initp.tile([g.BC, D], F32)
        nc.sync.dma_start(qr0[:], g.q_in[:, :])
        st_ps = g.tpps.tile([128, 16], F32, tag="tp", name="st_ps")
        for dh in range(2):
            nc.tensor.transpose(st_ps[:, dh * 8:dh * 8 + g.BC],
                                qr0[:, dh * 128:(dh + 1) * 128],
                                g.ident[0:g.BC, 0:g.BC])
        nc.vector.tensor_copy(g.stateT[:], st_ps[:])


def _prepass(g):
    """Normalize keys, transpose to keysT, cast fp16, fill resident kt tiles."""
    nc = g.nc
    with (
        g.tc.tile_pool(name="knatp", bufs=2) as knatp,
        g.tc.tile_pool(name="sqp", bufs=2) as sqp,
        g.tc.tile_pool(name="invp", bufs=2) as invp,
    ):
        drain_i = 0
        for bi in range(g.BC):
            for sl in range(g.NSLAB):
                knat = knatp.tile([128, g.SLC * 256], F32, tag="knat")
                src = g.k_in[bi, sl * g.SLAB:(sl + 1) * g.SLAB, :].rearrange(
                    "(c p) d -> p c d", p=128)
                nc.sync.dma_start(knat.rearrange("p (c d) -> p c d", d=256), src)
                inv = invp.tile([128, g.SLC], F32, tag="inv")
                for c in range(g.SLC):
                    kc = knat[:, c * 256:(c + 1) * 256]
                    sq = sqp.tile([128, 256], F32, tag="sq")
                    nc.scalar.activation(sq[:], kc, AF.Square,
                                         accum_out=inv[:, c:c + 1])
                nc.scalar.sqrt(inv[:], inv[:])
                nc.vector.tensor_scalar_max(inv[:], inv[:], 1e-12)
                nc.vector.reciprocal(inv[:], inv[:])
                for c in range(g.SLC):
                    kc = knat[:, c * 256:(c + 1) * 256]
                    nc.gpsimd.tensor_scalar_mul(kc, kc, inv[:, c:c + 1])
                for dh in range(2):
                    for cg in range(g.SLC // 4):
                        tp = g.tpps.tile([128, 512], F32, tag="tp", name="ktp_ps")
                        for ci in range(4):
                            c = cg * 4 + ci
                            nc.tensor.transpose(
                                tp[:, ci * 128:(ci + 1) * 128],
                                knat[:, c * 256 + dh * 128:c * 256 + dh * 128 + 128],
                                g.ident[:])
                        dst = g.kt[bi][dh][:, (sl * g.SLC + cg * 4) * 128:
                                           (sl * g.SLC + cg * 4 + 4) * 128]
                        nc.vector.tensor_copy(dst, tp[:])
                        drain_i += 1


def _hop_q(g, hh):
    """q = state @ Wq[hh] + bq[hh]; l2-normalize; transpose to qnT fp16."""
    nc = g.nc
    BC = g.BC
    bqt = g.scalp.tile([1, 500], F32, tag="biast", name="bqt")
    nc.scalar.dma_start(bqt[0:1, 0:D], g.bq_in[:, hh * D:(hh + 1) * D])
    q_ps = g.tpps.tile([BC, D], F32, tag="tp", name="q_ps")
    nc.tensor.matmul(q_ps[:, :], lhsT=g.ones[0:1, 0:BC], rhs=bqt[0:1, 0:D],
                     start=True, stop=False)
    for dh in range(2):
        wqt = g.wqp.tile([128, D], F32, tag="wqt")
        nc.sync.dma_start(wqt[:], g.wq_in[hh, dh])
        nc.tensor.matmul(q_ps[:, :], lhsT=g.stateT[:, dh * 8:dh * 8 + BC],
                         rhs=wqt[:], start=False, stop=(dh == 1))
    qtmp = g.scalp.tile([BC, D], F32, tag="qtmp", name="qsq")
    qss = g.scalp.tile([BC, 1], F32, tag="qss")
    nc.scalar.activation(qtmp[:], q_ps[:], AF.Square, accum_out=qss[:])
    nc.scalar.sqrt(qss[:], qss[:])
    nc.vector.tensor_scalar_max(qss[:], qss[:], 1e-12)
    nc.vector.reciprocal(qss[:], qss[:])
    qrows = g.scalp.tile([BC, D], F32, tag="qtmp", name="qrows")
    nc.vector.tensor_scalar_mul(qrows[:], q_ps[:], qss[:])
    qt_ps = g.tpps.tile([128, 16], F32, tag="tp", name="qt_ps")
    for dh in range(2):
        nc.tensor.transpose(qt_ps[:, dh * 8:dh * 8 + BC],
                            qrows[:, dh * 128:(dh + 1) * 128],
                            g.ident[0:BC, 0:BC])
    nc.vector.tensor_copy(g.qnT[:], qt_ps[:])
    # replicate each q column 32x so logits matmuls write full 128-partition
    # PSUM tiles (M=32 per col-group)
    for c in range(16):
        nc.vector.tensor_copy(g.qrep[:, c * 32:(c + 1) * 32],
                              g.qnT[:, c:c + 1].broadcast_to((128, 32)))


def _hop_logits_softmax(g, hh, pa, at):
    """One 4-batch pass: col-tiled logits matvecs, softmax, attn out+transpose."""
    nc = g.nc
    s = g.s
    expb = g.expp.tile([128, s], F32, tag="expb")
    parts = g.scalp.tile([128, g.NT], F32, tag="parts")
    for st in range(g.NT):
        lg = g.lgps.tile([128, 512], F32, tag="lg")
        for j in range(4):
            bi = pa * 4 + j
            for dh in range(2):
                nc.tensor.matmul(
                    lg[32 * j:32 * j + 32, :],
                    lhsT=g.qrep[:, (dh * 8 + bi) * 32:(dh * 8 + bi) * 32 + 32],
                    rhs=g.kt[bi][dh][:, st * 512:(st + 1) * 512],
                    start=(dh == 0), stop=(dh == 1),
                    tile_position=(0, 32 * j), skip_group_check=True)
        nc.scalar.activation(expb[:, st * 512:(st + 1) * 512], lg[:], AF.Exp,
                             accum_out=parts[:, st:st + 1])
    den = g.scalp.tile([128, 1], F32, tag="den")
    nc.vector.tensor_reduce(den[:], parts[:, 0:g.NT],
                            axis=mybir.AxisListType.X, op=AL.add)
    nc.vector.reciprocal(den[:], den[:])
    nc.vector.tensor_scalar_mul(expb[:], expb[:], den[:])
    for j in range(4):
        bi = pa * 4 + j
        nc.scalar.dma_start(g.attn_out[hh, bi, :], expb[32 * j:32 * j + 1, :])
    atv = at.rearrange("p (sc e) -> p sc e", e=8)
    for sc in range(g.SC):
        tp = g.tpps.tile([128, 128], F32, tag="tp", name="attp")
        nc.tensor.transpose(tp[:], expb[:, sc * 128:(sc + 1) * 128], g.ident[:])
        tpv = tp.rearrange("p (j r) -> p j r", j=4)
        if sc % 2 == 0:
            nc.vector.tensor_copy(atv[:, sc, pa * 4:pa * 4 + 4], tpv[:, :, 0])
        else:
            nc.scalar.copy(atv[:, sc, pa * 4:pa * 4 + 4], tpv[:, :, 0])


def _hop_vread(g, hh, pa, at):
    """Weighted V read for one 4-batch pass; update stateT."""
    nc = g.nc
    rd = g.rdps.tile([128, 256], F32, tag="rd")
    for vt_i in range(g.NVT):
        for jp in range(2):
            vts = []
            for j2 in range(2):
                j = jp * 2 + j2
                bi = pa * 4 + j
                vt = g.vp.tile([128, g.VCI * 256], F16, tag="vt")
                if hh == 0:
                    src = g.v_in[bi, vt_i * g.VCI * 128:(vt_i + 1) * g.VCI * 128, :]
                    src = src.rearrange("(c p) d -> p c d", p=128)
                    nc.gpsimd.dma_start(
                        vt.rearrange("p (c d) -> p c d", d=256), src)
                    nc.sync.dma_start(g.vcache[bi, vt_i], vt[:])
                else:
                    nc.sync.dma_start(vt[:], g.vcache[bi, vt_i])
                vts.append(vt)
            for ci in range(g.VCI):
                sc = vt_i * g.VCI + ci
                for j2 in range(2):
                    j = jp * 2 + j2
                    bi = pa * 4 + j
                    nc.tensor.matmul(
                        rd[32 * j:32 * j + 1, :],
                        lhsT=at[:, sc * 8 + bi:sc * 8 + bi + 1],
                        rhs=vts[j2][:, ci * 256:(ci + 1) * 256],
                        start=(sc == 0), stop=(sc == g.SC - 1),
                        tile_position=(0, 32 * j), skip_group_check=True)
    rdrows = g.rdrows
    for j in range(4):
        nc.vector.tensor_copy(rdrows[32 * j:32 * j + 1, :], rd[32 * j:32 * j + 1, :])
    rt = g.tpps.tile([128, 256], F32, tag="tp", name="rt")
    for dh in range(2):
        nc.tensor.transpose(rt[:, dh * 128:(dh + 1) * 128],
                            rdrows[:, dh * 128:(dh + 1) * 128], g.ident[:])
    rtv = rt.rearrange("p (dh j r) -> p dh j r", dh=2, j=4)
    for dh in range(2):
        sl = g.stateT[:, dh * 8 + pa * 4:dh * 8 + pa * 4 + 4]
        nc.vector.tensor_tensor(sl, sl, rtv[:, dh, :, 0], op=AL.add)


def _hop_gather(g, hh):
    """AllGather this hop's states across cores into a gst lhsT tile."""
    nc = g.nc
    b = g.b
    nc.sync.dma_start(g.cins[hh][:], g.stateT[:])
    if g.collective and g.n_cores > 1:
        nc.gpsimd.collective_compute(
            "AllGather", AL.bypass,
            replica_groups=[list(range(g.n_cores))],
            ins=[g.cins[hh][:]], outs=[g.couts[hh][:]])
    else:
        nc.sync.dma_start(g.couts[hh][0], g.cins[hh][:])
    gst = g.gstp.tile([128, 2 * b], F32, tag="gst")
    gstv = gst.rearrange("p (dh r j) -> p dh r j", dh=2, r=g.n_cores)
    srcv = g.couts[hh].rearrange("r p (dh j) -> p r dh j", dh=2)
    for dh in range(2):
        nc.gpsimd.dma_start(gstv[:, dh], srcv[:, :, dh])
    g.gsts[hh] = gst


def _hop_decode_compute(g, hh):
    """Decode hop hh logits for this core's E slice (uses g.gsts[hh])."""
    nc = g.nc
    b = g.b
    gst = g.gsts.pop(hh)
    for t in range(g.TD):
        bdt = g.scalp.tile([1, 500], F32, tag="biast", name="bdt")
        nc.scalar.dma_start(bdt[:], g.bd_in[:, hh * g.EC + t * 500:
                                            hh * g.EC + (t + 1) * 500])
        dec = g.decps.tile([128, 512], F32, tag="dec")
        nc.tensor.matmul(dec[0:b, 0:500], lhsT=g.ones[0:1, 0:b], rhs=bdt[:],
                         start=True, stop=False)
        for dh in range(2):
            wdt = g.wdp.tile([128, 500], F32, tag="wdt")
            nc.scalar.dma_start(wdt[:], g.wd_in[hh, dh, t])
            nc.tensor.matmul(dec[0:b, 0:500], lhsT=gst[:, dh * b:dh * b + b],
                             rhs=wdt[:], start=False, stop=(dh == 1))
        dout = g.outp.tile([b, 500], F32, tag="dout")
        nc.vector.tensor_copy(dout[:], dec[0:b, 0:500])
        nc.scalar.dma_start(g.logits_out[hh, :, t * 500:(t + 1) * 500], dout[:])


def _final_state(g):
    nc = g.nc
    so_ps = g.tpps.tile([16, 128], F32, tag="tp", name="so_ps")
    nc.tensor.transpose(so_ps[:], g.stateT[:], g.ident[:])
    ss = g.outp.tile([16, 128], F32, tag="dout", name="ss")
    nc.vector.tensor_copy(ss[:], so_ps[:])
    for dh in range(2):
        nc.scalar.dma_start(g.state_out[:, dh * 128:(dh + 1) * 128],
                            ss[dh * 8:dh * 8 + g.BC, :])


def build_program(n_cores=N_CORES, b=B, s=S, e=E, h_hops=H, collective=True, stage="full", reps=1):
    g = SimpleNamespace()
    g.n_cores, g.b, g.s, g.e, g.h_hops, g.collective = n_cores, b, s, e, h_hops, collective
    g.BC = b // n_cores           # batches per core
    g.EC = e // n_cores           # decoder entities per core
    g.SC = s // 128               # 128-row s-chunks
    g.NT = s // 512               # 512-wide logits tiles
    g.TD = g.EC // 500            # decoder n-tiles
    g.VCI = min(16, g.SC)         # 256-wide V chunks per V tile
    g.NVT = g.SC // g.VCI         # V tiles per batch
    g.SLAB = min(s, 2048)         # prepass slab (s rows)
    g.NSLAB = s // g.SLAB
    g.SLC = g.SLAB // 128         # chunks per slab

    nc = bacc.Bacc("TRN2", target_bir_lowering=False, debug=False,
                   num_devices=n_cores)
    g.nc = nc

    g.q_in = nc.dram_tensor("q_in", [g.BC, D], F32, kind="ExternalInput").ap()
    g.k_in = nc.dram_tensor("k_in", [g.BC, s, D], F32, kind="ExternalInput").ap()
    g.v_in = nc.dram_tensor("v_in", [g.BC, s, D], F32, kind="ExternalInput").ap()
    g.wq_in = nc.dram_tensor("wq_in", [h_hops, 2, 128, D], F32, kind="ExternalInput").ap()
    g.bq_in = nc.dram_tensor("bq_in", [1, h_hops * D], F32, kind="ExternalInput").ap()
    g.wd_in = nc.dram_tensor("wd_in", [h_hops, 2, g.TD, 128, 500], F32, kind="ExternalInput").ap()
    g.bd_in = nc.dram_tensor("bd_in", [1, h_hops * g.EC], F32, kind="ExternalInput").ap()
    g.state_out = nc.dram_tensor("state_out", [g.BC, D], F32, kind="ExternalOutput").ap()
    g.attn_out = nc.dram_tensor("attn_out", [h_hops, g.BC, s], F32, kind="ExternalOutput").ap()
    g.logits_out = nc.dram_tensor("logits_out", [h_hops, b, g.EC], F32, kind="ExternalOutput").ap()

    with tile.TileContext(nc) as tc, ExitStack() as ctx:
        g.tc = tc
        g.ctx = ctx
        consts = ctx.enter_context(tc.tile_pool(name="consts", bufs=1))
        ktp = ctx.enter_context(tc.tile_pool(name="ktp", bufs=1))
        statep = ctx.enter_context(tc.tile_pool(name="statep", bufs=1))
        dram = ctx.enter_context(tc.tile_pool(name="dram", bufs=1, space="DRAM"))
        g.lgps = ctx.enter_context(tc.tile_pool(name="lgps", bufs=3, space="PSUM"))
        g.tpps = ctx.enter_context(tc.tile_pool(name="tpps", bufs=2, space="PSUM"))
        g.rdps = ctx.enter_context(tc.tile_pool(name="rdps", bufs=2, space="PSUM"))
        g.decps = ctx.enter_context(tc.tile_pool(name="decps", bufs=1, space="PSUM"))

        g.ident = consts.tile([128, 128], F32, name="ident")
        masks.make_identity(nc, g.ident[:])
        g.ones = consts.tile([1, 64], F32, name="ones")
        nc.gpsimd.memset(g.ones[:], 1.0)

        g.kt = [[ktp.tile([128, s], F16, name=f"kt_{bi}_{dh}", tag=f"kt_{bi}_{dh}")
                 for dh in range(2)] for bi in range(g.BC)]
        g.stateT = statep.tile([128, 16], F32, name="stateT")
        g.qnT = statep.tile([128, 16], F16, name="qnT")
        g.qrep = statep.tile([128, 512], F16, name="qrep")
        g.rdrows = statep.tile([128, 256], F32, name="rdrows")
        nc.vector.memset(g.rdrows[:], 0.0)

        g.vcache = dram.tile([g.BC, g.NVT, 128, g.VCI * 256], F16, name="vcache")
        g.cins = [dram.tile([128, 16], F32, name=f"cin{hh}", tag=f"cin{hh}")
                  for hh in range(h_hops)]
        g.couts = [dram.tile([n_cores, 128, 16], F32, name=f"cout{hh}",
                             tag=f"cout{hh}",
                             addr_space="Shared" if collective else "Local")
                   for hh in range(h_hops)]

        # one-time PSUM init: ops below read full tiles whose unused lanes
        # are never written; mark every PSUM byte initialized once.
        for pool, tag, shape, n in ((g.lgps, "lg", [128, 512], 3),
                                    (g.tpps, "tp", [128, 512], 2),
                                    (g.rdps, "rd", [128, 256], 2),
                                    (g.decps, "dec", [128, 512], 1)):
            for _ in range(n):
                t = pool.tile(shape, F32, tag=tag, name="pz")
                nc.vector.memset(t[:], 0.0)

        for _rep in range(reps):
            _build_body(g, stage)

    nc.compile()
    return nc


def _build_body(g, stage):
        nc = g.nc
        tc = g.tc
        ctx = g.ctx
        _init_state(g)
        _prepass(g)

        if not hasattr(g, "wqp"):
            g.wqp = ctx.enter_context(tc.tile_pool(name="wqp", bufs=3))
            g.scalp = ctx.enter_context(tc.tile_pool(name="scalp", bufs=2))
            g.expp = ctx.enter_context(tc.tile_pool(name="expp", bufs=2))
            g.atp = ctx.enter_context(tc.tile_pool(name="atp", bufs=2))
            g.vp = ctx.enter_context(tc.tile_pool(name="vp", bufs=3))
            g.wdp = ctx.enter_context(tc.tile_pool(name="wdp", bufs=2))
            g.gstp = ctx.enter_context(tc.tile_pool(name="gstp", bufs=2))
            g.outp = ctx.enter_context(tc.tile_pool(name="op", bufs=2))

        if stage != "prepass":
            g.gsts = {}
            for hh in range(g.h_hops):
                _hop_q(g, hh)
                at = g.atp.tile([128, g.SC * 8], F16, tag="at", name="at")
                for pa in range(2):
                    _hop_logits_softmax(g, hh, pa, at)
                if stage == "logits":
                    continue
                if stage == "full" and hh > 0:
                    _hop_decode_compute(g, hh - 1)
                for pa in range(2):
                    _hop_vread(g, hh, pa, at)
                if stage == "vread":
                    continue
                _hop_gather(g, hh)
            if stage == "full":
                _hop_decode_compute(g, g.h_hops - 1)
        _final_state(g)



# ---------------------------------------------------------------------------
# host wrapper
# ---------------------------------------------------------------------------

_RUNNER = {}


def make_in_maps(query, keys, values, Wq, bq, Wd, bd, n_cores=N_CORES,
                 b=B, e=E, h_hops=H):
    BC = b // n_cores
    EC = e // n_cores
    TD = EC // 500
    wq_h = np.ascontiguousarray(Wq.reshape(h_hops, 2, 128, D))
    bq_h = np.ascontiguousarray(bq.reshape(1, h_hops * D))
    in_maps = []
    for c in range(n_cores):
        bs = slice(c * BC, (c + 1) * BC)
        es = slice(c * EC, (c + 1) * EC)
        wd_c = Wd[:, :, es].reshape(h_hops, 2, 128, TD, 500).transpose(0, 1, 3, 2, 4)
        in_maps.append({
            "q_in": np.ascontiguousarray(query[bs]),
            "k_in": np.ascontiguousarray(keys[bs]),
            "v_in": np.ascontiguousarray(values[bs]),
            "wq_in": wq_h,
            "bq_in": bq_h,
            "wd_in": np.ascontiguousarray(wd_c),
            "bd_in": np.ascontiguousarray(bd[:, es].reshape(1, h_hops * EC)),
        })
    return in_maps


def assemble(results):
    state = np.concatenate([r["state_out"] for r in results], axis=0)
    attn = np.concatenate([r["attn_out"] for r in results], axis=1)
    logits = np.concatenate([r["logits_out"] for r in results], axis=2)
    return state, logits, attn


def get_runner():
    """Build+compile the program once; return a callable in_maps -> results."""
    if "fn" in _RUNNER:
        return _RUNNER["fn"]
    import jax
    from jax.sharding import Mesh, PartitionSpec
    from jax.experimental.shard_map import shard_map
    from concourse import bass2jax

    nc = build_program()
    bass2jax.install_neuronx_cc_hook()

    partition_name = (nc.partition_id_tensor.name
                      if nc.partition_id_tensor else None)
    in_names, out_names, out_avals, zero_outs = [], [], [], []
    for alloc in nc.m.functions[0].allocations:
        if not isinstance(alloc, mybir.MemoryLocationSet):
            continue
        name = alloc.memorylocations[0].name
        if alloc.kind == "ExternalInput":
            if name != partition_name:
                in_names.append(name)
        elif alloc.kind == "ExternalOutput":
            out_names.append(name)
            shape = tuple(alloc.tensor_shape)
            dtype = mybir.dt.np(alloc.dtype)
            out_avals.append(jax.core.ShapedArray(shape, dtype))
            zero_outs.append(np.zeros(shape, dtype))
    n_params = len(in_names)
    all_in_names = in_names + out_names
    if partition_name is not None:
        all_in_names = all_in_names + [partition_name]

    def _body(*args):
        operands = list(args)
        if partition_name is not None:
            operands.append(bass2jax.partition_id_tensor())
        outs = bass2jax._bass_exec_p.bind(
            *operands,
            out_avals=tuple(out_avals),
            in_names=tuple(all_in_names),
            out_names=tuple(out_names),
            lowering_input_output_aliases=(),
            sim_require_finite=False,
            sim_require_nnan=False,
            nc=nc,
        )
        return tuple(outs)

    devices = jax.devices()[:N_CORES]
    mesh = Mesh(np.asarray(devices), ("core",))
    n_outs = len(out_names)
    sharded = jax.jit(
        shard_map(_body, mesh=mesh,
                  in_specs=(PartitionSpec("core"),) * (n_params + n_outs),
                  out_specs=(PartitionSpec("core"),) * n_outs,
                  check_rep=False),
        donate_argnums=tuple(range(n_params, n_params + n_outs)),
        keep_unused=True,
    )

    def fn(in_maps):
        concat_in = [
            np.concatenate([np.asarray(in_maps[c][nm]) for c in range(N_CORES)], axis=0)
            for nm in in_names
        ]
        concat_zeros = [
            np.zeros((N_CORES * z.shape[0], *z.shape[1:]), z.dtype) for z in zero_outs
        ]
        out_arrs = sharded(*concat_in, *concat_zeros)
        return [
            {nm: np.asarray(out_arrs[i]).reshape(N_CORES, *out_avals[i].shape)[c]
             for i, nm in enumerate(out_names)}
            for c in range(N_CORES)
        ]

    fn.sharded = sharded
    fn.in_names = in_names
    fn.out_names = out_names
    fn.out_avals = out_avals
    fn.zero_outs = zero_outs
    fn.mesh = mesh
    _RUNNER["fn"] = fn
    return fn


def kernel(query, keys, values, Wq, bq, Wd, bd):
    query, keys, values, Wq, bq, Wd, bd = [
        np.asarray(x, dtype=np.float32)
        for x in (query, keys, values, Wq, bq, Wd, bd)
    ]
    fn = get_runner()
    results = fn(make_in_maps(query, keys, values, Wq, bq, Wd, bd))
    return assemble(results)
